# revision 47
# baseline (speedup 1.0000x reference)
"""Trainium2 Bass kernel for BaseAttention (Bahdanau-style additive attention).

Reference computation (per batch row b):
    att_h  = h @ W.T + b_h                         # [B, A]
    dot    = tanh(iaf + att_h[:, None, :])         # [B, L, A]
    scores = dot @ alpha + alpha_b                 # [B, L]
    w      = softmax(scores, axis=1)               # [B, L]
    out    = sum_l w[b, l] * af[b, l, :]           # [B, D]

Sharding: data-parallel over batch, B=128 -> 16 per core across 8 cores.

The kernel is HBM-bandwidth bound; the big streamed tensors (af, iaf, W) are
downcast to fp16 on the host, halving DMA bytes (rel tolerance is 2e-2; fp16
keeps us ~3e-4).  Per-core layout (rows = (b, l) flattened, R=3136):

  - af and iaf are pre-tiled on the host to [P, NT, *] so every DMA descriptor
    is a large contiguous run (16KB for af groups, 4KB+ for iaf chunks).
  - the tensor engine is power-throttled to ~1.2GHz while DMA streams, so PE
    work during the stream is minimized: per 4-tile af group the 16 N=512
    weighted-sum matmuls are emitted in *reversed* tile order so the first
    matmul's semaphore wait covers the whole group's e-columns (accumulation
    is commutative; start/stop sit on the first/last emitted matmul) and the
    rest run back-to-back with no waits.
  - the softmax denominator needs no per-tile matmul: e-columns are reduced
    over tiles with one DVE op at the end plus a single N=1 matmul.
  - scores via one fused DVE scalar_tensor_tensor: (tanh*1)*alpha with
    accum_out giving the row sums directly; exp batched per group; att_h
    broadcast matmuls staged a group ahead so their PSUM waits resolve off
    the critical path.
"""

from contextlib import ExitStack

import numpy as np

import concourse.bass as bass
import concourse.mybir as mybir
import concourse.tile as tile
from concourse import bacc
from concourse.bass_utils import run_bass_kernel_spmd

F32 = mybir.dt.float32
F16 = mybir.dt.float16
AF_T = mybir.ActivationFunctionType
ALU = mybir.AluOpType

B, L, D, A = 128, 196, 2048, 512
NCORES = 8
BPC = B // NCORES          # 16 batch rows per core
R = BPC * L                # 3136 (b, l) rows per core
P = 128                    # partitions
NT = (R + P - 1) // P      # 25 row tiles (24 full + one 64-row tail)
KCH = D // P               # 16 k-chunks for the h @ W.T matmul
WCH = 4                    # W DMA chunks (earlier att_h start)
DCH = 4                    # d chunks of 512 for the weighted sum
DC = D // DCH              # 512
AFG = 4                    # row tiles per streamed DMA group
TAILR = R - (NT - 1) * P   # 64 rows in the last tile

GROUPS = [(t0, min(AFG, NT - t0)) for t0 in range(0, NT, AFG)]
# iaf chunk issue schedule: {group index -> (tile0, ntiles)}; each chunk must
# be issued before any emit_bc() that reads its tiles (tiles 0-7 go up front)
IAF_CHUNKS = {0: (8, 4), 1: (12, 4), 2: (16, 4), 3: (20, 4), 4: (24, 1)}


def _build_program():
    nc = bacc.Bacc(None, target_bir_lowering=False)

    h_t = nc.declare_dram_parameter("h_t", [P, KCH * BPC], F16, isOutput=False)
    w_t = nc.declare_dram_parameter("w_t", [D, A], F16, isOutput=False)
    b_bc = nc.declare_dram_parameter("b_bc", [BPC, A], F16, isOutput=False)
    alpha_bc = nc.declare_dram_parameter("alpha_bc", [P, A], F16, isOutput=False)
    alphab_bc = nc.declare_dram_parameter("alphab_bc", [P, 1], F32, isOutput=False)
    ind = nc.declare_dram_parameter("ind", [P, NT * BPC], F16, isOutput=False)
    ind_t = nc.declare_dram_parameter("ind_t", [BPC, R], F16, isOutput=False)
    ident = nc.declare_dram_parameter("ident", [P, P], F16, isOutput=False)
    iaf = nc.declare_dram_parameter("iaf", [P, NT * A], F16, isOutput=False)
    af = nc.declare_dram_parameter("af", [P, NT * D], F16, isOutput=False)
    out = nc.declare_dram_parameter("out", [BPC, D], F32, isOutput=True)

    with ExitStack() as ctx:
        tc = ctx.enter_context(tile.TileContext(nc))
        consts = ctx.enter_context(tc.tile_pool(name="consts", bufs=1))
        wpool = ctx.enter_context(tc.tile_pool(name="wpool", bufs=1))
        iafp = ctx.enter_context(tc.tile_pool(name="iafp", bufs=1))
        afp = ctx.enter_context(tc.tile_pool(name="afp", bufs=5))
        scr = ctx.enter_context(tc.tile_pool(name="scr", bufs=4))
        ps_bc = ctx.enter_context(
            tc.tile_pool(name="ps_bc", bufs=3, space=bass.MemorySpace.PSUM)
        )
        ps_hb = ctx.enter_context(
            tc.tile_pool(name="ps_hb", bufs=1, space=bass.MemorySpace.PSUM)
        )
        ps_acc = ctx.enter_context(
            tc.tile_pool(name="ps_acc", bufs=1, space=bass.MemorySpace.PSUM)
        )


        # --- h and W head the queue: everything downstream gates on att_h,
        # so their bytes must land first; W in chunks so the accumulation
        # starts while W still streams ---
        ht_sb = consts.tile([P, KCH, BPC], F16)
        nc.sync.dma_start(ht_sb[:], h_t[:, :].rearrange("p (k b) -> p k b", k=KCH))
        w_sb = wpool.tile([P, KCH, A], F16)
        KPW = KCH // WCH
        for wc in range(WCH):
            nc.sync.dma_start(
                w_sb[:, wc * KPW : (wc + 1) * KPW, :],
                w_t[wc * KPW * P : (wc + 1) * KPW * P, :].rearrange(
                    "(k p) a -> p k a", p=P
                ),
            )
        bbc_sb = consts.tile([BPC, A], F16)
        nc.sync.dma_start(bbc_sb[:], b_bc[:, :])
        indt_sb = consts.tile([BPC, R], F16)
        nc.sync.dma_start(indt_sb[:], ind_t[:, :])
        ident_sb = consts.tile([P, P], F16)
        nc.sync.dma_start(ident_sb[:], ident[:, :])
        abc_sb = consts.tile([P, A], F16)
        nc.sync.dma_start(abc_sb[:], alpha_bc[:, :])
        abb_sb = consts.tile([P, 1], F32)
        nc.sync.dma_start(abb_sb[:], alphab_bc[:, :])
        ind_sb = consts.tile([P, NT, BPC], F16)
        nc.sync.dma_start(ind_sb[:], ind[:, :].rearrange("p (t b) -> p t b", t=NT))

        af_tiles = {}
        iaf_all = iafp.tile([P, NT, A], F16)

        def issue_iaf(t0, n):
            nc.sync.dma_start(
                iaf_all[:, t0 : t0 + n, :],
                iaf[:, t0 * A : (t0 + n) * A].rearrange("p (t a) -> p t a", t=n),
            )

        scores_all = consts.tile([P, NT], F32)
        e_all = consts.tile([P, NT], F32)
        ecols_all = consts.tile([P, NT, BPC], F16)
        # tail-tile rows never written by the scores pipeline must be zero for
        # the end-of-kernel denominator reduce
        nc.vector.memset(ecols_all[TAILR:, NT - 1, :], 0.0)
        ered = consts.tile([P, BPC], F32)
        ones1 = consts.tile([P, 1], F32)
        nc.vector.memset(ones1[:], 1.0)

        # --- att_hb = h @ W.T + b_h, shape [BPC, A] ---
        atthb_ps = ps_hb.tile([BPC, A], F32)
        for k in range(KCH):
            nc.tensor.matmul(
                atthb_ps[:],
                ht_sb[:, k, :],
                w_sb[:, k, :],
                start=(k == 0),
                stop=(k == KCH - 1),
            )
        atthb_sb = consts.tile([BPC, A], F16)
        nc.vector.tensor_add(atthb_sb[:], atthb_ps[:], bbc_sb[:])

        # --- weighted-sum accumulator ---
        acc_ps = ps_acc.tile([BPC, DCH, DC], F32)

        bc_tiles = {}

        def emit_bc(t):
            """x_t = broadcast(att_h) + iaf_t, built fully inside PSUM: a
            row-select matmul then an identity-stationary matmul accumulating
            the iaf tile, so no vector-engine add is needed at all."""
            if t >= NT or t in bc_tiles:
                return
            pt = P if t < NT - 1 else TAILR
            rt = t * P
            bc_ps = ps_bc.tile([P, A], F32, tag="bc")
            nc.tensor.matmul(
                bc_ps[:pt, :],
                indt_sb[:, rt : rt + pt],
                atthb_sb[:],
                start=True,
                stop=False,
            )
            nc.tensor.matmul(
                bc_ps[:pt, :],
                ident_sb[:pt, :pt],
                iaf_all[:pt, t, :],
                start=False,
                stop=True,
            )
            bc_tiles[t] = bc_ps

        issue_iaf(0, 8)
        for t in range(6):
            emit_bc(t)

        # dependency-free filler matmuls into the dead atthb PSUM bank: they
        # keep the tensor engine executing across inter-group idle gaps so its
        # clock stays ramped; the next burst's semaphore wait absorbs them
        fill_ps = atthb_ps

        def emit_fillers(k):
            for _ in range(k):
                nc.tensor.matmul(
                    fill_ps[:], ind_sb[:, 0, :], abc_sb[:, :], start=True, stop=True
                )

        for gi, (t0, n) in enumerate(GROUPS):
            tiles = [(t, P if t < NT - 1 else TAILR) for t in range(t0, t0 + n)]

            # --- stream DMAs ---
            if gi in IAF_CHUNKS:
                issue_iaf(*IAF_CHUNKS[gi])
            af_g = afp.tile([P, AFG, D], F16, tag="af")
            nc.sync.dma_start(
                af_g[:, :n, :],
                af[:, t0 * D : (t0 + n) * D].rearrange("p (t d) -> p t d", t=n),
            )
            for jj in range(n):
                af_tiles[t0 + jj] = (af_g, jj)

            # --- scores chains (scalar + DVE), batched per stage ---
            tanhs = {}
            for t, pt in tiles:
                tanh = scr.tile([P, A], F16, tag="tanh")
                nc.scalar.activation(
                    tanh[:pt, :], bc_tiles.pop(t)[:pt, :], AF_T.Tanh
                )
                tanhs[t] = tanh
            for t, pt in tiles:
                junk = scr.tile([P, A], F16, tag="junk")
                nc.vector.scalar_tensor_tensor(
                    junk[:pt, :],
                    tanhs[t][:pt, :],
                    1.0,
                    abc_sb[:pt, :],
                    op0=ALU.mult,
                    op1=ALU.mult,
                    accum_out=scores_all[:pt, t : t + 1],
                )
            gpt = tiles[-1][1] if t0 + n == NT else P
            nc.scalar.activation(
                e_all[:gpt, t0 : t0 + n],
                scores_all[:gpt, t0 : t0 + n],
                AF_T.Exp,
                bias=abb_sb[:gpt, :],
            )
            for t, pt in tiles:
                nc.vector.tensor_scalar_mul(
                    ecols_all[:pt, t, :], ind_sb[:pt, t, :], e_all[:pt, t : t + 1]
                )
            if t0 + n == NT - 1:
                # partial denominator over tiles 0..23 as soon as they exist,
                # so only tile 24's contribution remains on the end chain
                nc.vector.tensor_reduce(
                    ered[:, :],
                    ecols_all[:, : NT - 1, :].rearrange("p t b -> p b t"),
                    axis=mybir.AxisListType.X,
                    op=ALU.add,
                )

            # the next group's broadcasts all precede the burst (the PSUM
            # rotation frees on this group's tanh reads), so the next scores
            # chain fully overlaps the burst
            for t in range(t0 + 4, t0 + 8):
                emit_bc(t)

            # --- PE: weighted-sum matmuls, reversed so one ecols wait covers
            # the group and the rest run back-to-back ---
            for j, t in enumerate(reversed(range(t0, t0 + n))):
                pt = P if t < NT - 1 else TAILR
                af_g, af_j = af_tiles.pop(t)
                for c in range(DCH):
                    nc.tensor.matmul(
                        acc_ps[:, c, :],
                        ecols_all[:pt, t, :],
                        af_g[:pt, af_j, c * DC : (c + 1) * DC],
                        start=(gi == 0 and j == 0),
                        stop=(t == NT - 1 and c == DCH - 1),
                    )
            # fillers keep the PE clock ramped across early inter-group gaps;
            # late in the stream the chain is the constraint, so skip them
            if gi < 3:
                emit_fillers(3)

        # --- denominator: fold in the last tile, one N=1 matmul, reciprocal ---
        ered2 = consts.tile([P, BPC], F32)
        nc.vector.tensor_add(ered2[:, :], ered[:, :], ecols_all[:, NT - 1, :])
        sums_ps = ps_bc.tile([BPC, 1], F32, tag="bc")
        nc.tensor.matmul(sums_ps[:], ered2[:, :], ones1[:], start=True, stop=True)
        recip = consts.tile([BPC, 1], F32)
        nc.vector.reciprocal(recip[:], sums_ps[:])

        # --- normalize (scalar handles the low half, vector the high half)
        # and store each half as soon as it is ready ---
        out_sb = consts.tile([BPC, D], F32)
        nc.scalar.mul(out_sb[:, 0:DC], acc_ps[:, 0, :], recip[:])
        nc.vector.tensor_scalar_mul(out_sb[:, 2 * DC : 3 * DC], acc_ps[:, 2, :], recip[:])
        nc.scalar.mul(out_sb[:, DC : 2 * DC], acc_ps[:, 1, :], recip[:])
        nc.vector.tensor_scalar_mul(out_sb[:, 3 * DC :], acc_ps[:, 3, :], recip[:])
        nc.sync.dma_start(out[:, : 2 * DC], out_sb[:, : 2 * DC])
        nc.sync.dma_start(out[:, 2 * DC :], out_sb[:, 2 * DC :])

    nc.compile()
    return nc


_PROGRAM = None


def _get_program():
    global _PROGRAM
    if _PROGRAM is None:
        _PROGRAM = _build_program()
    return _PROGRAM


def _tile_rows(x, width):
    """[R, width] -> [P, NT*width] with row r at partition r%P, tile r//P."""
    padded = np.zeros((NT * P, width), np.float16)
    padded[:R] = x
    return np.ascontiguousarray(
        padded.reshape(NT, P, width).transpose(1, 0, 2).reshape(P, NT * width)
    )


def _host_prep(h, att_feats, internal_att_feats, h2att_w, h2att_b, alpha_w, alpha_b):
    h16 = np.asarray(h, np.float32).astype(np.float16)
    af16 = np.asarray(att_feats, np.float32).astype(np.float16)
    iaf16 = np.asarray(internal_att_feats, np.float32).astype(np.float16)
    h2att_w = np.asarray(h2att_w, np.float32)
    h2att_b = np.asarray(h2att_b, np.float32)
    alpha_w = np.asarray(alpha_w, np.float32)
    alpha_b = np.asarray(alpha_b, np.float32)

    w_t = np.ascontiguousarray(h2att_w.T.astype(np.float16))   # [D, A]
    b_bc = np.tile(h2att_b.reshape(1, A).astype(np.float16), (BPC, 1))
    alpha_bc = np.tile(alpha_w.reshape(1, A).astype(np.float16), (P, 1))
    alphab_bc = np.full((P, 1), float(alpha_b.reshape(-1)[0]), np.float32)

    ind = np.zeros((R, BPC), np.float16)
    rows = np.arange(R)
    ind[rows, rows // L] = 1.0
    ind_t = np.ascontiguousarray(ind.T)                        # [BPC, R]
    ind_tiled = _tile_rows(ind, BPC)                           # [P, NT*BPC]

    in_maps = []
    for i in range(NCORES):
        sl = slice(i * BPC, (i + 1) * BPC)
        h_t = h16[sl].T                                        # [D, BPC]
        in_maps.append(
            {
                "h_t": np.ascontiguousarray(
                    h_t.reshape(KCH, P, BPC).transpose(1, 0, 2).reshape(P, KCH * BPC)
                ),
                "w_t": w_t,
                "b_bc": b_bc,
                "alpha_bc": alpha_bc,
                "alphab_bc": alphab_bc,
                "ind": ind_tiled,
                "ind_t": ind_t,
                "ident": np.eye(P, dtype=np.float16),
                "iaf": _tile_rows(iaf16[sl].reshape(R, A), A),
                "af": _tile_rows(af16[sl].reshape(R, D), D),
            }
        )
    return in_maps


def run(trace=False, **inputs):
    """Run the SPMD kernel; returns (full_output [B, D], BassKernelResults)."""
    nc = _get_program()
    in_maps = _host_prep(**inputs)
    res = run_bass_kernel_spmd(nc, in_maps, list(range(NCORES)), trace=trace)
    out = np.concatenate([res.results[i]["out"] for i in range(NCORES)], axis=0)
    return out, res


def kernel(**inputs):
    out, _ = run(trace=False, **inputs)
    return out


# revision 48
# speedup vs baseline: 1.0437x; 1.0437x over previous
"""Trainium2 Bass kernel for BaseAttention (Bahdanau-style additive attention).

Reference computation (per batch row b):
    att_h  = h @ W.T + b_h                         # [B, A]
    dot    = tanh(iaf + att_h[:, None, :])         # [B, L, A]
    scores = dot @ alpha + alpha_b                 # [B, L]
    w      = softmax(scores, axis=1)               # [B, L]
    out    = sum_l w[b, l] * af[b, l, :]           # [B, D]

Sharding: data-parallel over batch, B=128 -> 16 per core across 8 cores.

The kernel is HBM-bandwidth bound; the big streamed tensors (af, iaf, W) are
downcast to fp16 on the host, halving DMA bytes (rel tolerance is 2e-2; fp16
keeps us ~3e-4).  Per-core layout (rows = (b, l) flattened, R=3136):

  - af and iaf are pre-tiled on the host to [P, NT, *] so every DMA descriptor
    is a large contiguous run (16KB for af groups, 4KB+ for iaf chunks).
  - the tensor engine is power-throttled to ~1.2GHz while DMA streams, so PE
    work during the stream is minimized: per 4-tile af group the 16 N=512
    weighted-sum matmuls are emitted in *reversed* tile order so the first
    matmul's semaphore wait covers the whole group's e-columns (accumulation
    is commutative; start/stop sit on the first/last emitted matmul) and the
    rest run back-to-back with no waits.
  - the softmax denominator needs no per-tile matmul: e-columns are reduced
    over tiles with one DVE op at the end plus a single N=1 matmul.
  - scores via one fused DVE scalar_tensor_tensor: (tanh*1)*alpha with
    accum_out giving the row sums directly; exp batched per group; att_h
    broadcast matmuls staged a group ahead so their PSUM waits resolve off
    the critical path.
"""

from contextlib import ExitStack

import numpy as np

import concourse.bass as bass
import concourse.mybir as mybir
import concourse.tile as tile
from concourse import bacc
from concourse.bass_utils import run_bass_kernel_spmd

F32 = mybir.dt.float32
F16 = mybir.dt.float16
AF_T = mybir.ActivationFunctionType
ALU = mybir.AluOpType

B, L, D, A = 128, 196, 2048, 512
NCORES = 8
BPC = B // NCORES          # 16 batch rows per core
R = BPC * L                # 3136 (b, l) rows per core
P = 128                    # partitions
NT = (R + P - 1) // P      # 25 row tiles (24 full + one 64-row tail)
KCH = D // P               # 16 k-chunks for the h @ W.T matmul
WCH = 4                    # W DMA chunks (earlier att_h start)
DCH = 4                    # d chunks of 512 for the weighted sum
DC = D // DCH              # 512
AFG = 4                    # row tiles per streamed DMA group
TAILR = R - (NT - 1) * P   # 64 rows in the last tile

GROUPS = [(t0, min(AFG, NT - t0)) for t0 in range(0, NT, AFG)]
# iaf chunk issue schedule: {group index -> (tile0, ntiles)}; each chunk must
# be issued before any emit_bc() that reads its tiles (tiles 0-7 go up front)
IAF_CHUNKS = {0: (8, 4), 1: (12, 4), 2: (16, 4), 3: (20, 4), 4: (24, 1)}


def _build_program():
    nc = bacc.Bacc(None, target_bir_lowering=False)

    h_t = nc.declare_dram_parameter("h_t", [P, KCH * BPC], F16, isOutput=False)
    w_t = nc.declare_dram_parameter("w_t", [D, A], F16, isOutput=False)
    b_bc = nc.declare_dram_parameter("b_bc", [BPC, A], F16, isOutput=False)
    alpha_bc = nc.declare_dram_parameter("alpha_bc", [P, A], F16, isOutput=False)
    alphab_bc = nc.declare_dram_parameter("alphab_bc", [P, 1], F32, isOutput=False)
    ind = nc.declare_dram_parameter("ind", [P, NT * BPC], F16, isOutput=False)
    ind_t = nc.declare_dram_parameter("ind_t", [BPC, R], F16, isOutput=False)
    ident = nc.declare_dram_parameter("ident", [P, P], F16, isOutput=False)
    iaf = nc.declare_dram_parameter("iaf", [P, NT * A], F16, isOutput=False)
    af = nc.declare_dram_parameter("af", [P, NT * D], F16, isOutput=False)
    out = nc.declare_dram_parameter("out", [BPC, D], F32, isOutput=True)

    with ExitStack() as ctx:
        tc = ctx.enter_context(tile.TileContext(nc))
        consts = ctx.enter_context(tc.tile_pool(name="consts", bufs=1))
        wpool = ctx.enter_context(tc.tile_pool(name="wpool", bufs=1))
        iafp = ctx.enter_context(tc.tile_pool(name="iafp", bufs=1))
        afp = ctx.enter_context(tc.tile_pool(name="afp", bufs=5))
        scr = ctx.enter_context(tc.tile_pool(name="scr", bufs=4))
        ps_bc = ctx.enter_context(
            tc.tile_pool(name="ps_bc", bufs=3, space=bass.MemorySpace.PSUM)
        )
        ps_hb = ctx.enter_context(
            tc.tile_pool(name="ps_hb", bufs=1, space=bass.MemorySpace.PSUM)
        )
        ps_acc = ctx.enter_context(
            tc.tile_pool(name="ps_acc", bufs=1, space=bass.MemorySpace.PSUM)
        )


        # --- small constants ---
        ht_sb = consts.tile([P, KCH, BPC], F16)
        nc.sync.dma_start(ht_sb[:], h_t[:, :].rearrange("p (k b) -> p k b", k=KCH))
        bbc_sb = consts.tile([BPC, A], F16)
        nc.sync.dma_start(bbc_sb[:], b_bc[:, :])
        indt_sb = consts.tile([BPC, R], F16)
        nc.sync.dma_start(indt_sb[:], ind_t[:, :])
        abc_sb = consts.tile([P, A], F16)
        nc.sync.dma_start(abc_sb[:], alpha_bc[:, :])
        abb_sb = consts.tile([P, 1], F32)
        nc.sync.dma_start(abb_sb[:], alphab_bc[:, :])
        ind_sb = consts.tile([P, NT, BPC], F16)
        nc.sync.dma_start(ind_sb[:], ind[:, :].rearrange("p (t b) -> p t b", t=NT))
        ident_sb = consts.tile([P, P], F16)
        nc.sync.dma_start(ident_sb[:], ident[:, :])

        # --- W in chunks so att_h accumulation starts while W streams ---
        w_sb = wpool.tile([P, KCH, A], F16)
        KPW = KCH // WCH
        for wc in range(WCH):
            nc.sync.dma_start(
                w_sb[:, wc * KPW : (wc + 1) * KPW, :],
                w_t[wc * KPW * P : (wc + 1) * KPW * P, :].rearrange(
                    "(k p) a -> p k a", p=P
                ),
            )

        af_tiles = {}
        iaf_all = iafp.tile([P, NT, A], F16)

        def issue_iaf(t0, n):
            nc.sync.dma_start(
                iaf_all[:, t0 : t0 + n, :],
                iaf[:, t0 * A : (t0 + n) * A].rearrange("p (t a) -> p t a", t=n),
            )

        scores_all = consts.tile([P, NT], F32)
        e_all = consts.tile([P, NT], F32)
        ecols_all = consts.tile([P, NT, BPC], F16)
        # tail-tile rows never written by the scores pipeline must be zero for
        # the end-of-kernel denominator reduce
        nc.vector.memset(ecols_all[TAILR:, NT - 1, :], 0.0)

        # --- att_hb = h @ W.T + b_h, shape [BPC, A] ---
        atthb_ps = ps_hb.tile([BPC, A], F32)
        for k in range(KCH):
            nc.tensor.matmul(
                atthb_ps[:],
                ht_sb[:, k, :],
                w_sb[:, k, :],
                start=(k == 0),
                stop=(k == KCH - 1),
            )
        atthb_sb = consts.tile([BPC, A], F16)
        nc.vector.tensor_add(atthb_sb[:], atthb_ps[:], bbc_sb[:])

        # --- weighted-sum accumulator ---
        acc_ps = ps_acc.tile([BPC, DCH, DC], F32)

        bc_tiles = {}

        def emit_bc(t):
            """x_t = broadcast(att_h) + iaf_t, built fully inside PSUM: a
            row-select matmul then an identity-stationary matmul accumulating
            the iaf tile, so no vector-engine add is needed at all."""
            if t >= NT or t in bc_tiles:
                return
            pt = P if t < NT - 1 else TAILR
            rt = t * P
            bc_ps = ps_bc.tile([P, A], F32, tag="bc")
            nc.tensor.matmul(
                bc_ps[:pt, :],
                indt_sb[:, rt : rt + pt],
                atthb_sb[:],
                start=True,
                stop=False,
            )
            nc.tensor.matmul(
                bc_ps[:pt, :],
                ident_sb[:pt, :pt],
                iaf_all[:pt, t, :],
                start=False,
                stop=True,
            )
            bc_tiles[t] = bc_ps

        issue_iaf(0, 8)
        for t in range(6):
            emit_bc(t)

        # dependency-free filler matmuls into the dead atthb PSUM bank: they
        # keep the tensor engine executing across inter-group idle gaps so its
        # clock stays ramped; the next burst's semaphore wait absorbs them
        fill_ps = atthb_ps

        def emit_fillers(k):
            for _ in range(k):
                nc.tensor.matmul(
                    fill_ps[:], ind_sb[:, 0, :], abc_sb[:, :], start=True, stop=True
                )

        for gi, (t0, n) in enumerate(GROUPS):
            tiles = [(t, P if t < NT - 1 else TAILR) for t in range(t0, t0 + n)]

            # --- stream DMAs ---
            if gi in IAF_CHUNKS:
                issue_iaf(*IAF_CHUNKS[gi])
            af_g = afp.tile([P, AFG, D], F16, tag="af")
            nc.sync.dma_start(
                af_g[:, :n, :],
                af[:, t0 * D : (t0 + n) * D].rearrange("p (t d) -> p t d", t=n),
            )
            for jj in range(n):
                af_tiles[t0 + jj] = (af_g, jj)

            # --- scores chains (scalar + DVE), batched per stage ---
            tanhs = {}
            for t, pt in tiles:
                tanh = scr.tile([P, A], F16, tag="tanh")
                nc.scalar.activation(
                    tanh[:pt, :], bc_tiles.pop(t)[:pt, :], AF_T.Tanh
                )
                tanhs[t] = tanh
            for t, pt in tiles:
                junk = scr.tile([P, A], F16, tag="junk")
                nc.vector.scalar_tensor_tensor(
                    junk[:pt, :],
                    tanhs[t][:pt, :],
                    1.0,
                    abc_sb[:pt, :],
                    op0=ALU.mult,
                    op1=ALU.mult,
                    accum_out=scores_all[:pt, t : t + 1],
                )
            gpt = tiles[-1][1] if t0 + n == NT else P
            nc.scalar.activation(
                e_all[:gpt, t0 : t0 + n],
                scores_all[:gpt, t0 : t0 + n],
                AF_T.Exp,
                bias=abb_sb[:gpt, :],
            )
            for t, pt in tiles:
                nc.vector.tensor_scalar_mul(
                    ecols_all[:pt, t, :], ind_sb[:pt, t, :], e_all[:pt, t : t + 1]
                )

            # the next group's broadcasts all precede the burst (the PSUM
            # rotation frees on this group's tanh reads), so the next scores
            # chain fully overlaps the burst
            for t in range(t0 + 4, t0 + 8):
                emit_bc(t)

            # --- PE: weighted-sum matmuls, reversed so one ecols wait covers
            # the group and the rest run back-to-back ---
            for j, t in enumerate(reversed(range(t0, t0 + n))):
                pt = P if t < NT - 1 else TAILR
                af_g, af_j = af_tiles.pop(t)
                for c in range(DCH):
                    nc.tensor.matmul(
                        acc_ps[:, c, :],
                        ecols_all[:pt, t, :],
                        af_g[:pt, af_j, c * DC : (c + 1) * DC],
                        start=(gi == 0 and j == 0),
                        stop=(t == NT - 1 and c == DCH - 1),
                    )
            # fillers keep the PE clock ramped across early inter-group gaps;
            # late in the stream the chain is the constraint, so skip them
            if gi < 3:
                emit_fillers(3)

        # --- denominator: one reduce over tiles + one N=1 matmul ---
        ered = consts.tile([P, BPC], F32)
        nc.vector.tensor_reduce(
            ered[:, :],
            ecols_all[:, :, :].rearrange("p t b -> p b t"),
            axis=mybir.AxisListType.X,
            op=ALU.add,
        )
        ones1 = consts.tile([P, 1], F32)
        nc.vector.memset(ones1[:], 1.0)
        sums_ps = ps_bc.tile([BPC, 1], F32, tag="bc")
        nc.tensor.matmul(sums_ps[:], ered[:, :], ones1[:], start=True, stop=True)
        recip = consts.tile([BPC, 1], F32)
        nc.vector.reciprocal(recip[:], sums_ps[:])

        # --- normalize (split scalar/vector) and store ---
        out_sb = consts.tile([BPC, D], F32)
        for c in range(DCH):
            dst = out_sb[:, c * DC : (c + 1) * DC]
            if c < 2:
                nc.scalar.mul(dst, acc_ps[:, c, :], recip[:])
            else:
                nc.vector.tensor_scalar_mul(dst, acc_ps[:, c, :], recip[:])
        nc.sync.dma_start(out[:, :], out_sb[:])

    nc.compile()
    return nc


_PROGRAM = None


def _get_program():
    global _PROGRAM
    if _PROGRAM is None:
        _PROGRAM = _build_program()
    return _PROGRAM


def _tile_rows(x, width):
    """[R, width] -> [P, NT*width] with row r at partition r%P, tile r//P."""
    padded = np.zeros((NT * P, width), np.float16)
    padded[:R] = x
    return np.ascontiguousarray(
        padded.reshape(NT, P, width).transpose(1, 0, 2).reshape(P, NT * width)
    )


def _host_prep(h, att_feats, internal_att_feats, h2att_w, h2att_b, alpha_w, alpha_b):
    h16 = np.asarray(h, np.float32).astype(np.float16)
    af16 = np.asarray(att_feats, np.float32).astype(np.float16)
    iaf16 = np.asarray(internal_att_feats, np.float32).astype(np.float16)
    h2att_w = np.asarray(h2att_w, np.float32)
    h2att_b = np.asarray(h2att_b, np.float32)
    alpha_w = np.asarray(alpha_w, np.float32)
    alpha_b = np.asarray(alpha_b, np.float32)

    w_t = np.ascontiguousarray(h2att_w.T.astype(np.float16))   # [D, A]
    b_bc = np.tile(h2att_b.reshape(1, A).astype(np.float16), (BPC, 1))
    alpha_bc = np.tile(alpha_w.reshape(1, A).astype(np.float16), (P, 1))
    alphab_bc = np.full((P, 1), float(alpha_b.reshape(-1)[0]), np.float32)

    ind = np.zeros((R, BPC), np.float16)
    rows = np.arange(R)
    ind[rows, rows // L] = 1.0
    ind_t = np.ascontiguousarray(ind.T)                        # [BPC, R]
    ind_tiled = _tile_rows(ind, BPC)                           # [P, NT*BPC]

    in_maps = []
    for i in range(NCORES):
        sl = slice(i * BPC, (i + 1) * BPC)
        h_t = h16[sl].T                                        # [D, BPC]
        in_maps.append(
            {
                "h_t": np.ascontiguousarray(
                    h_t.reshape(KCH, P, BPC).transpose(1, 0, 2).reshape(P, KCH * BPC)
                ),
                "w_t": w_t,
                "b_bc": b_bc,
                "alpha_bc": alpha_bc,
                "alphab_bc": alphab_bc,
                "ind": ind_tiled,
                "ind_t": ind_t,
                "ident": np.eye(P, dtype=np.float16),
                "iaf": _tile_rows(iaf16[sl].reshape(R, A), A),
                "af": _tile_rows(af16[sl].reshape(R, D), D),
            }
        )
    return in_maps


def run(trace=False, **inputs):
    """Run the SPMD kernel; returns (full_output [B, D], BassKernelResults)."""
    nc = _get_program()
    in_maps = _host_prep(**inputs)
    res = run_bass_kernel_spmd(nc, in_maps, list(range(NCORES)), trace=trace)
    out = np.concatenate([res.results[i]["out"] for i in range(NCORES)], axis=0)
    return out, res


def kernel(**inputs):
    out, _ = run(trace=False, **inputs)
    return out


# revision 49
# speedup vs baseline: 1.0575x; 1.0132x over previous
"""Trainium2 Bass kernel for BaseAttention (Bahdanau-style additive attention).

Reference computation (per batch row b):
    att_h  = h @ W.T + b_h                         # [B, A]
    dot    = tanh(iaf + att_h[:, None, :])         # [B, L, A]
    scores = dot @ alpha + alpha_b                 # [B, L]
    w      = softmax(scores, axis=1)               # [B, L]
    out    = sum_l w[b, l] * af[b, l, :]           # [B, D]

Sharding: data-parallel over batch, B=128 -> 16 per core across 8 cores.

The kernel is HBM-bandwidth bound; the big streamed tensors (af, iaf, W) are
downcast to fp16 on the host, halving DMA bytes (rel tolerance is 2e-2; fp16
keeps us ~3e-4).  Per-core layout (rows = (b, l) flattened, R=3136):

  - af and iaf are pre-tiled on the host to [P, NT, *] so every DMA descriptor
    is a large contiguous run (16KB for af groups, 4KB+ for iaf chunks).
  - the tensor engine is power-throttled to ~1.2GHz while DMA streams, so PE
    work during the stream is minimized: per 4-tile af group the 16 N=512
    weighted-sum matmuls are emitted in *reversed* tile order so the first
    matmul's semaphore wait covers the whole group's e-columns (accumulation
    is commutative; start/stop sit on the first/last emitted matmul) and the
    rest run back-to-back with no waits.
  - the softmax denominator needs no per-tile matmul: e-columns are reduced
    over tiles with one DVE op at the end plus a single N=1 matmul.
  - scores via one fused DVE scalar_tensor_tensor: (tanh*1)*alpha with
    accum_out giving the row sums directly; exp batched per group; att_h
    broadcast matmuls staged a group ahead so their PSUM waits resolve off
    the critical path.
"""

from contextlib import ExitStack

import numpy as np

import concourse.bass as bass
import concourse.mybir as mybir
import concourse.tile as tile
from concourse import bacc
from concourse.bass_utils import run_bass_kernel_spmd

F32 = mybir.dt.float32
F16 = mybir.dt.float16
AF_T = mybir.ActivationFunctionType
ALU = mybir.AluOpType

B, L, D, A = 128, 196, 2048, 512
NCORES = 8
BPC = B // NCORES          # 16 batch rows per core
R = BPC * L                # 3136 (b, l) rows per core
P = 128                    # partitions
NT = (R + P - 1) // P      # 25 row tiles (24 full + one 64-row tail)
KCH = D // P               # 16 k-chunks for the h @ W.T matmul
WCH = 4                    # W DMA chunks (earlier att_h start)
DCH = 4                    # d chunks of 512 for the weighted sum
DC = D // DCH              # 512
AFG = 4                    # row tiles per streamed DMA group
TAILR = R - (NT - 1) * P   # 64 rows in the last tile

GROUPS = [(t0, min(AFG, NT - t0)) for t0 in range(0, NT, AFG)]
# iaf chunk issue schedule: {group index -> (tile0, ntiles)}; each chunk must
# be issued before any emit_bc() that reads its tiles (tiles 0-7 go up front)
IAF_CHUNKS = {0: (8, 4), 1: (12, 4), 2: (16, 4), 3: (20, 4), 4: (24, 1)}


def _build_program():
    nc = bacc.Bacc(None, target_bir_lowering=False)

    h_t = nc.declare_dram_parameter("h_t", [P, KCH * BPC], F16, isOutput=False)
    w_t = nc.declare_dram_parameter("w_t", [D, A], F16, isOutput=False)
    b_bc = nc.declare_dram_parameter("b_bc", [BPC, A], F16, isOutput=False)
    alpha_bc = nc.declare_dram_parameter("alpha_bc", [P, A], F16, isOutput=False)
    alphab_bc = nc.declare_dram_parameter("alphab_bc", [P, 1], F32, isOutput=False)
    ind = nc.declare_dram_parameter("ind", [P, NT * BPC], F16, isOutput=False)
    ind_t = nc.declare_dram_parameter("ind_t", [BPC, R], F16, isOutput=False)
    ident = nc.declare_dram_parameter("ident", [P, P], F16, isOutput=False)
    iaf = nc.declare_dram_parameter("iaf", [P, NT * A], F16, isOutput=False)
    af = nc.declare_dram_parameter("af", [P, NT * D], F16, isOutput=False)
    out = nc.declare_dram_parameter("out", [BPC, D], F32, isOutput=True)

    with ExitStack() as ctx:
        tc = ctx.enter_context(tile.TileContext(nc))
        consts = ctx.enter_context(tc.tile_pool(name="consts", bufs=1))
        wpool = ctx.enter_context(tc.tile_pool(name="wpool", bufs=1))
        iafp = ctx.enter_context(tc.tile_pool(name="iafp", bufs=1))
        afp = ctx.enter_context(tc.tile_pool(name="afp", bufs=5))
        scr = ctx.enter_context(tc.tile_pool(name="scr", bufs=4))
        ps_bc = ctx.enter_context(
            tc.tile_pool(name="ps_bc", bufs=3, space=bass.MemorySpace.PSUM)
        )
        ps_hb = ctx.enter_context(
            tc.tile_pool(name="ps_hb", bufs=1, space=bass.MemorySpace.PSUM)
        )
        ps_acc = ctx.enter_context(
            tc.tile_pool(name="ps_acc", bufs=1, space=bass.MemorySpace.PSUM)
        )


        # --- h and W head the DMA queue: att_h gates every scores chain, so
        # its inputs must land first; W in chunks so the accumulation starts
        # while W still streams ---
        ht_sb = consts.tile([P, KCH, BPC], F16)
        nc.sync.dma_start(ht_sb[:], h_t[:, :].rearrange("p (k b) -> p k b", k=KCH))
        w_sb = wpool.tile([P, KCH, A], F16)
        KPW = KCH // WCH
        for wc in range(WCH):
            nc.sync.dma_start(
                w_sb[:, wc * KPW : (wc + 1) * KPW, :],
                w_t[wc * KPW * P : (wc + 1) * KPW * P, :].rearrange(
                    "(k p) a -> p k a", p=P
                ),
            )

        # --- remaining small constants ---
        bbc_sb = consts.tile([BPC, A], F16)
        nc.sync.dma_start(bbc_sb[:], b_bc[:, :])
        indt_sb = consts.tile([BPC, R], F16)
        nc.sync.dma_start(indt_sb[:], ind_t[:, :])
        ident_sb = consts.tile([P, P], F16)
        nc.sync.dma_start(ident_sb[:], ident[:, :])
        abc_sb = consts.tile([P, A], F16)
        nc.sync.dma_start(abc_sb[:], alpha_bc[:, :])
        abb_sb = consts.tile([P, 1], F32)
        nc.sync.dma_start(abb_sb[:], alphab_bc[:, :])
        ind_sb = consts.tile([P, NT, BPC], F16)
        nc.sync.dma_start(ind_sb[:], ind[:, :].rearrange("p (t b) -> p t b", t=NT))

        af_tiles = {}
        iaf_all = iafp.tile([P, NT, A], F16)

        def issue_iaf(t0, n):
            nc.sync.dma_start(
                iaf_all[:, t0 : t0 + n, :],
                iaf[:, t0 * A : (t0 + n) * A].rearrange("p (t a) -> p t a", t=n),
            )

        scores_all = consts.tile([P, NT], F32)
        e_all = consts.tile([P, NT], F32)
        ecols_all = consts.tile([P, NT, BPC], F16)
        # tail-tile rows never written by the scores pipeline must be zero for
        # the end-of-kernel denominator reduce
        nc.vector.memset(ecols_all[TAILR:, NT - 1, :], 0.0)

        # --- att_hb = h @ W.T + b_h, shape [BPC, A] ---
        atthb_ps = ps_hb.tile([BPC, A], F32)
        for k in range(KCH):
            nc.tensor.matmul(
                atthb_ps[:],
                ht_sb[:, k, :],
                w_sb[:, k, :],
                start=(k == 0),
                stop=(k == KCH - 1),
            )
        atthb_sb = consts.tile([BPC, A], F16)
        nc.vector.tensor_add(atthb_sb[:], atthb_ps[:], bbc_sb[:])

        # --- weighted-sum accumulator ---
        acc_ps = ps_acc.tile([BPC, DCH, DC], F32)

        bc_tiles = {}

        def emit_bc(t):
            """x_t = broadcast(att_h) + iaf_t, built fully inside PSUM: a
            row-select matmul then an identity-stationary matmul accumulating
            the iaf tile, so no vector-engine add is needed at all."""
            if t >= NT or t in bc_tiles:
                return
            pt = P if t < NT - 1 else TAILR
            rt = t * P
            bc_ps = ps_bc.tile([P, A], F32, tag="bc")
            nc.tensor.matmul(
                bc_ps[:pt, :],
                indt_sb[:, rt : rt + pt],
                atthb_sb[:],
                start=True,
                stop=False,
            )
            nc.tensor.matmul(
                bc_ps[:pt, :],
                ident_sb[:pt, :pt],
                iaf_all[:pt, t, :],
                start=False,
                stop=True,
            )
            bc_tiles[t] = bc_ps

        issue_iaf(0, 8)
        for t in range(6):
            emit_bc(t)

        # dependency-free filler matmuls into the dead atthb PSUM bank: they
        # keep the tensor engine executing across inter-group idle gaps so its
        # clock stays ramped; the next burst's semaphore wait absorbs them
        fill_ps = atthb_ps

        def emit_fillers(k):
            for _ in range(k):
                nc.tensor.matmul(
                    fill_ps[:], ind_sb[:, 0, :], abc_sb[:, :], start=True, stop=True
                )

        for gi, (t0, n) in enumerate(GROUPS):
            tiles = [(t, P if t < NT - 1 else TAILR) for t in range(t0, t0 + n)]

            # --- stream DMAs ---
            if gi in IAF_CHUNKS:
                issue_iaf(*IAF_CHUNKS[gi])
            af_g = afp.tile([P, AFG, D], F16, tag="af")
            nc.sync.dma_start(
                af_g[:, :n, :],
                af[:, t0 * D : (t0 + n) * D].rearrange("p (t d) -> p t d", t=n),
            )
            for jj in range(n):
                af_tiles[t0 + jj] = (af_g, jj)

            # --- scores chains (scalar + DVE), batched per stage ---
            tanhs = {}
            for t, pt in tiles:
                tanh = scr.tile([P, A], F16, tag="tanh")
                nc.scalar.activation(
                    tanh[:pt, :], bc_tiles.pop(t)[:pt, :], AF_T.Tanh
                )
                tanhs[t] = tanh
            for t, pt in tiles:
                junk = scr.tile([P, A], F16, tag="junk")
                nc.vector.scalar_tensor_tensor(
                    junk[:pt, :],
                    tanhs[t][:pt, :],
                    1.0,
                    abc_sb[:pt, :],
                    op0=ALU.mult,
                    op1=ALU.mult,
                    accum_out=scores_all[:pt, t : t + 1],
                )
            gpt = tiles[-1][1] if t0 + n == NT else P
            nc.scalar.activation(
                e_all[:gpt, t0 : t0 + n],
                scores_all[:gpt, t0 : t0 + n],
                AF_T.Exp,
                bias=abb_sb[:gpt, :],
            )
            for t, pt in tiles:
                nc.vector.tensor_scalar_mul(
                    ecols_all[:pt, t, :], ind_sb[:pt, t, :], e_all[:pt, t : t + 1]
                )

            # the next group's broadcasts all precede the burst (the PSUM
            # rotation frees on this group's tanh reads), so the next scores
            # chain fully overlaps the burst
            for t in range(t0 + 4, t0 + 8):
                emit_bc(t)

            # --- PE: weighted-sum matmuls, reversed so one ecols wait covers
            # the group and the rest run back-to-back ---
            for j, t in enumerate(reversed(range(t0, t0 + n))):
                pt = P if t < NT - 1 else TAILR
                af_g, af_j = af_tiles.pop(t)
                for c in range(DCH):
                    nc.tensor.matmul(
                        acc_ps[:, c, :],
                        ecols_all[:pt, t, :],
                        af_g[:pt, af_j, c * DC : (c + 1) * DC],
                        start=(gi == 0 and j == 0),
                        stop=(t == NT - 1 and c == DCH - 1),
                    )
            # fillers keep the PE clock ramped across early inter-group gaps;
            # late in the stream the chain is the constraint, so skip them
            if gi < 3:
                emit_fillers(3)

        # --- denominator: one reduce over tiles + one N=1 matmul ---
        ered = consts.tile([P, BPC], F32)
        nc.vector.tensor_reduce(
            ered[:, :],
            ecols_all[:, :, :].rearrange("p t b -> p b t"),
            axis=mybir.AxisListType.X,
            op=ALU.add,
        )
        ones1 = consts.tile([P, 1], F32)
        nc.vector.memset(ones1[:], 1.0)
        sums_ps = ps_bc.tile([BPC, 1], F32, tag="bc")
        nc.tensor.matmul(sums_ps[:], ered[:, :], ones1[:], start=True, stop=True)
        recip = consts.tile([BPC, 1], F32)
        nc.vector.reciprocal(recip[:], sums_ps[:])

        # --- normalize (split scalar/vector) and store ---
        out_sb = consts.tile([BPC, D], F32)
        for c in range(DCH):
            dst = out_sb[:, c * DC : (c + 1) * DC]
            if c < 2:
                nc.scalar.mul(dst, acc_ps[:, c, :], recip[:])
            else:
                nc.vector.tensor_scalar_mul(dst, acc_ps[:, c, :], recip[:])
        nc.sync.dma_start(out[:, :], out_sb[:])

    nc.compile()
    return nc


_PROGRAM = None


def _get_program():
    global _PROGRAM
    if _PROGRAM is None:
        _PROGRAM = _build_program()
    return _PROGRAM


def _tile_rows(x, width):
    """[R, width] -> [P, NT*width] with row r at partition r%P, tile r//P."""
    padded = np.zeros((NT * P, width), np.float16)
    padded[:R] = x
    return np.ascontiguousarray(
        padded.reshape(NT, P, width).transpose(1, 0, 2).reshape(P, NT * width)
    )


def _host_prep(h, att_feats, internal_att_feats, h2att_w, h2att_b, alpha_w, alpha_b):
    h16 = np.asarray(h, np.float32).astype(np.float16)
    af16 = np.asarray(att_feats, np.float32).astype(np.float16)
    iaf16 = np.asarray(internal_att_feats, np.float32).astype(np.float16)
    h2att_w = np.asarray(h2att_w, np.float32)
    h2att_b = np.asarray(h2att_b, np.float32)
    alpha_w = np.asarray(alpha_w, np.float32)
    alpha_b = np.asarray(alpha_b, np.float32)

    w_t = np.ascontiguousarray(h2att_w.T.astype(np.float16))   # [D, A]
    b_bc = np.tile(h2att_b.reshape(1, A).astype(np.float16), (BPC, 1))
    alpha_bc = np.tile(alpha_w.reshape(1, A).astype(np.float16), (P, 1))
    alphab_bc = np.full((P, 1), float(alpha_b.reshape(-1)[0]), np.float32)

    ind = np.zeros((R, BPC), np.float16)
    rows = np.arange(R)
    ind[rows, rows // L] = 1.0
    ind_t = np.ascontiguousarray(ind.T)                        # [BPC, R]
    ind_tiled = _tile_rows(ind, BPC)                           # [P, NT*BPC]

    in_maps = []
    for i in range(NCORES):
        sl = slice(i * BPC, (i + 1) * BPC)
        h_t = h16[sl].T                                        # [D, BPC]
        in_maps.append(
            {
                "h_t": np.ascontiguousarray(
                    h_t.reshape(KCH, P, BPC).transpose(1, 0, 2).reshape(P, KCH * BPC)
                ),
                "w_t": w_t,
                "b_bc": b_bc,
                "alpha_bc": alpha_bc,
                "alphab_bc": alphab_bc,
                "ind": ind_tiled,
                "ind_t": ind_t,
                "ident": np.eye(P, dtype=np.float16),
                "iaf": _tile_rows(iaf16[sl].reshape(R, A), A),
                "af": _tile_rows(af16[sl].reshape(R, D), D),
            }
        )
    return in_maps


def run(trace=False, **inputs):
    """Run the SPMD kernel; returns (full_output [B, D], BassKernelResults)."""
    nc = _get_program()
    in_maps = _host_prep(**inputs)
    res = run_bass_kernel_spmd(nc, in_maps, list(range(NCORES)), trace=trace)
    out = np.concatenate([res.results[i]["out"] for i in range(NCORES)], axis=0)
    return out, res


def kernel(**inputs):
    out, _ = run(trace=False, **inputs)
    return out


# revision 50
# speedup vs baseline: 1.0692x; 1.0111x over previous
"""Trainium2 Bass kernel for BaseAttention (Bahdanau-style additive attention).

Reference computation (per batch row b):
    att_h  = h @ W.T + b_h                         # [B, A]
    dot    = tanh(iaf + att_h[:, None, :])         # [B, L, A]
    scores = dot @ alpha + alpha_b                 # [B, L]
    w      = softmax(scores, axis=1)               # [B, L]
    out    = sum_l w[b, l] * af[b, l, :]           # [B, D]

Sharding: data-parallel over batch, B=128 -> 16 per core across 8 cores.

The kernel is HBM-bandwidth bound; the big streamed tensors (af, iaf, W) are
downcast to fp16 on the host, halving DMA bytes (rel tolerance is 2e-2; fp16
keeps us ~3e-4).  Per-core layout (rows = (b, l) flattened, R=3136):

  - af and iaf are pre-tiled on the host to [P, NT, *] so every DMA descriptor
    is a large contiguous run (16KB for af groups, 4KB+ for iaf chunks).
  - the tensor engine is power-throttled to ~1.2GHz while DMA streams, so PE
    work during the stream is minimized: per 4-tile af group the 16 N=512
    weighted-sum matmuls are emitted in *reversed* tile order so the first
    matmul's semaphore wait covers the whole group's e-columns (accumulation
    is commutative; start/stop sit on the first/last emitted matmul) and the
    rest run back-to-back with no waits.
  - the softmax denominator needs no per-tile matmul: e-columns are reduced
    over tiles with one DVE op at the end plus a single N=1 matmul.
  - scores via one fused DVE scalar_tensor_tensor: (tanh*1)*alpha with
    accum_out giving the row sums directly; exp batched per group; att_h
    broadcast matmuls staged a group ahead so their PSUM waits resolve off
    the critical path.
"""

from contextlib import ExitStack

import numpy as np

import concourse.bass as bass
import concourse.mybir as mybir
import concourse.tile as tile
from concourse import bacc
from concourse.bass_utils import run_bass_kernel_spmd

F32 = mybir.dt.float32
F16 = mybir.dt.float16
AF_T = mybir.ActivationFunctionType
ALU = mybir.AluOpType

B, L, D, A = 128, 196, 2048, 512
NCORES = 8
BPC = B // NCORES          # 16 batch rows per core
R = BPC * L                # 3136 (b, l) rows per core
P = 128                    # partitions
NT = (R + P - 1) // P      # 25 row tiles (24 full + one 64-row tail)
KCH = D // P               # 16 k-chunks for the h @ W.T matmul
WCH = 4                    # W DMA chunks (earlier att_h start)
DCH = 4                    # d chunks of 512 for the weighted sum
DC = D // DCH              # 512
AFG = 4                    # row tiles per streamed DMA group
TAILR = R - (NT - 1) * P   # 64 rows in the last tile

GROUPS = [(t0, min(AFG, NT - t0)) for t0 in range(0, NT, AFG)]
# iaf chunk issue schedule: {group index -> (tile0, ntiles)}; each chunk must
# be issued before any emit_bc() that reads its tiles (tiles 0-7 go up front)
IAF_CHUNKS = {0: (8, 4), 1: (12, 4), 2: (16, 4), 3: (20, 4), 4: (24, 1)}


def _build_program():
    nc = bacc.Bacc(None, target_bir_lowering=False)

    h_t = nc.declare_dram_parameter("h_t", [P, KCH * BPC], F16, isOutput=False)
    w_t = nc.declare_dram_parameter("w_t", [D, A], F16, isOutput=False)
    b_bc = nc.declare_dram_parameter("b_bc", [BPC, A], F16, isOutput=False)
    alpha_bc = nc.declare_dram_parameter("alpha_bc", [P, A], F16, isOutput=False)
    alphab_bc = nc.declare_dram_parameter("alphab_bc", [P, 1], F32, isOutput=False)
    ind = nc.declare_dram_parameter("ind", [P, NT * BPC], F16, isOutput=False)
    ind_t = nc.declare_dram_parameter("ind_t", [BPC, R], F16, isOutput=False)
    ident = nc.declare_dram_parameter("ident", [P, P], F16, isOutput=False)
    iaf = nc.declare_dram_parameter("iaf", [P, NT * A], F16, isOutput=False)
    af = nc.declare_dram_parameter("af", [P, NT * D], F16, isOutput=False)
    out = nc.declare_dram_parameter("out", [BPC, D], F32, isOutput=True)

    with ExitStack() as ctx:
        tc = ctx.enter_context(tile.TileContext(nc))
        consts = ctx.enter_context(tc.tile_pool(name="consts", bufs=1))
        wpool = ctx.enter_context(tc.tile_pool(name="wpool", bufs=1))
        iafp = ctx.enter_context(tc.tile_pool(name="iafp", bufs=1))
        afp = ctx.enter_context(tc.tile_pool(name="afp", bufs=6))
        scr = ctx.enter_context(tc.tile_pool(name="scr", bufs=4))
        ps_bc = ctx.enter_context(
            tc.tile_pool(name="ps_bc", bufs=3, space=bass.MemorySpace.PSUM)
        )
        ps_hb = ctx.enter_context(
            tc.tile_pool(name="ps_hb", bufs=1, space=bass.MemorySpace.PSUM)
        )
        ps_acc = ctx.enter_context(
            tc.tile_pool(name="ps_acc", bufs=1, space=bass.MemorySpace.PSUM)
        )


        # --- h and W head the DMA queue: att_h gates every scores chain, so
        # its inputs must land first; W in chunks so the accumulation starts
        # while W still streams ---
        ht_sb = consts.tile([P, KCH, BPC], F16)
        nc.sync.dma_start(ht_sb[:], h_t[:, :].rearrange("p (k b) -> p k b", k=KCH))
        w_sb = wpool.tile([P, KCH, A], F16)
        KPW = KCH // WCH
        for wc in range(WCH):
            nc.sync.dma_start(
                w_sb[:, wc * KPW : (wc + 1) * KPW, :],
                w_t[wc * KPW * P : (wc + 1) * KPW * P, :].rearrange(
                    "(k p) a -> p k a", p=P
                ),
            )

        # --- remaining small constants ---
        bbc_sb = consts.tile([BPC, A], F16)
        nc.sync.dma_start(bbc_sb[:], b_bc[:, :])
        indt_sb = consts.tile([BPC, R], F16)
        nc.sync.dma_start(indt_sb[:], ind_t[:, :])
        ident_sb = consts.tile([P, P], F16)
        nc.sync.dma_start(ident_sb[:], ident[:, :])
        abc_sb = consts.tile([P, A], F16)
        nc.sync.dma_start(abc_sb[:], alpha_bc[:, :])
        abb_sb = consts.tile([P, 1], F32)
        nc.sync.dma_start(abb_sb[:], alphab_bc[:, :])
        ind_sb = consts.tile([P, NT, BPC], F16)
        nc.sync.dma_start(ind_sb[:], ind[:, :].rearrange("p (t b) -> p t b", t=NT))

        af_tiles = {}
        iaf_all = iafp.tile([P, NT, A], F16)

        def issue_iaf(t0, n):
            nc.sync.dma_start(
                iaf_all[:, t0 : t0 + n, :],
                iaf[:, t0 * A : (t0 + n) * A].rearrange("p (t a) -> p t a", t=n),
            )

        scores_all = consts.tile([P, NT], F32)
        e_all = consts.tile([P, NT], F32)
        ecols_all = consts.tile([P, NT, BPC], F16)
        # tail-tile rows never written by the scores pipeline must be zero for
        # the end-of-kernel denominator reduce
        nc.vector.memset(ecols_all[TAILR:, NT - 1, :], 0.0)

        # --- att_hb = h @ W.T + b_h, shape [BPC, A] ---
        atthb_ps = ps_hb.tile([BPC, A], F32)
        for k in range(KCH):
            nc.tensor.matmul(
                atthb_ps[:],
                ht_sb[:, k, :],
                w_sb[:, k, :],
                start=(k == 0),
                stop=(k == KCH - 1),
            )
        atthb_sb = consts.tile([BPC, A], F16)
        nc.vector.tensor_add(atthb_sb[:], atthb_ps[:], bbc_sb[:])

        # --- weighted-sum accumulator ---
        acc_ps = ps_acc.tile([BPC, DCH, DC], F32)

        bc_tiles = {}

        def emit_bc(t):
            """x_t = broadcast(att_h) + iaf_t, built fully inside PSUM: a
            row-select matmul then an identity-stationary matmul accumulating
            the iaf tile, so no vector-engine add is needed at all."""
            if t >= NT or t in bc_tiles:
                return
            pt = P if t < NT - 1 else TAILR
            rt = t * P
            bc_ps = ps_bc.tile([P, A], F32, tag="bc")
            nc.tensor.matmul(
                bc_ps[:pt, :],
                indt_sb[:, rt : rt + pt],
                atthb_sb[:],
                start=True,
                stop=False,
            )
            nc.tensor.matmul(
                bc_ps[:pt, :],
                ident_sb[:pt, :pt],
                iaf_all[:pt, t, :],
                start=False,
                stop=True,
            )
            bc_tiles[t] = bc_ps

        issue_iaf(0, 8)
        for t in range(6):
            emit_bc(t)

        # dependency-free filler matmuls into the dead atthb PSUM bank: they
        # keep the tensor engine executing across inter-group idle gaps so its
        # clock stays ramped; the next burst's semaphore wait absorbs them
        fill_ps = atthb_ps

        def emit_fillers(k):
            for _ in range(k):
                nc.tensor.matmul(
                    fill_ps[:], ind_sb[:, 0, :], abc_sb[:, :], start=True, stop=True
                )

        for gi, (t0, n) in enumerate(GROUPS):
            tiles = [(t, P if t < NT - 1 else TAILR) for t in range(t0, t0 + n)]

            # --- stream DMAs ---
            if gi in IAF_CHUNKS:
                issue_iaf(*IAF_CHUNKS[gi])
            af_g = afp.tile([P, AFG, D], F16, tag="af")
            nc.sync.dma_start(
                af_g[:, :n, :],
                af[:, t0 * D : (t0 + n) * D].rearrange("p (t d) -> p t d", t=n),
            )
            for jj in range(n):
                af_tiles[t0 + jj] = (af_g, jj)

            # --- scores chains (scalar + DVE), batched per stage ---
            tanhs = {}
            for t, pt in tiles:
                tanh = scr.tile([P, A], F16, tag="tanh")
                nc.scalar.activation(
                    tanh[:pt, :], bc_tiles.pop(t)[:pt, :], AF_T.Tanh
                )
                tanhs[t] = tanh
            for t, pt in tiles:
                junk = scr.tile([P, A], F16, tag="junk")
                nc.vector.scalar_tensor_tensor(
                    junk[:pt, :],
                    tanhs[t][:pt, :],
                    1.0,
                    abc_sb[:pt, :],
                    op0=ALU.mult,
                    op1=ALU.mult,
                    accum_out=scores_all[:pt, t : t + 1],
                )
            gpt = tiles[-1][1] if t0 + n == NT else P
            nc.scalar.activation(
                e_all[:gpt, t0 : t0 + n],
                scores_all[:gpt, t0 : t0 + n],
                AF_T.Exp,
                bias=abb_sb[:gpt, :],
            )
            for t, pt in tiles:
                nc.vector.tensor_scalar_mul(
                    ecols_all[:pt, t, :], ind_sb[:pt, t, :], e_all[:pt, t : t + 1]
                )

            # the next group's broadcasts all precede the burst (the PSUM
            # rotation frees on this group's tanh reads), so the next scores
            # chain fully overlaps the burst
            for t in range(t0 + 4, t0 + 8):
                emit_bc(t)

            # --- PE: weighted-sum matmuls, reversed so one ecols wait covers
            # the group and the rest run back-to-back ---
            for j, t in enumerate(reversed(range(t0, t0 + n))):
                pt = P if t < NT - 1 else TAILR
                af_g, af_j = af_tiles.pop(t)
                for c in range(DCH):
                    nc.tensor.matmul(
                        acc_ps[:, c, :],
                        ecols_all[:pt, t, :],
                        af_g[:pt, af_j, c * DC : (c + 1) * DC],
                        start=(gi == 0 and j == 0),
                        stop=(t == NT - 1 and c == DCH - 1),
                    )
            # fillers keep the PE clock ramped across early inter-group gaps;
            # late in the stream the chain is the constraint, so skip them
            if gi < 3:
                emit_fillers(3)

        # --- denominator: one reduce over tiles + one N=1 matmul ---
        ered = consts.tile([P, BPC], F32)
        nc.vector.tensor_reduce(
            ered[:, :],
            ecols_all[:, :, :].rearrange("p t b -> p b t"),
            axis=mybir.AxisListType.X,
            op=ALU.add,
        )
        ones1 = consts.tile([P, 1], F32)
        nc.vector.memset(ones1[:], 1.0)
        sums_ps = ps_bc.tile([BPC, 1], F32, tag="bc")
        nc.tensor.matmul(sums_ps[:], ered[:, :], ones1[:], start=True, stop=True)
        recip = consts.tile([BPC, 1], F32)
        nc.vector.reciprocal(recip[:], sums_ps[:])

        # --- normalize (split scalar/vector) and store ---
        out_sb = consts.tile([BPC, D], F32)
        for c in range(DCH):
            dst = out_sb[:, c * DC : (c + 1) * DC]
            if c < 2:
                nc.scalar.mul(dst, acc_ps[:, c, :], recip[:])
            else:
                nc.vector.tensor_scalar_mul(dst, acc_ps[:, c, :], recip[:])
        nc.sync.dma_start(out[:, :], out_sb[:])

    nc.compile()
    return nc


_PROGRAM = None


def _get_program():
    global _PROGRAM
    if _PROGRAM is None:
        _PROGRAM = _build_program()
    return _PROGRAM


def _tile_rows(x, width):
    """[R, width] -> [P, NT*width] with row r at partition r%P, tile r//P."""
    padded = np.zeros((NT * P, width), np.float16)
    padded[:R] = x
    return np.ascontiguousarray(
        padded.reshape(NT, P, width).transpose(1, 0, 2).reshape(P, NT * width)
    )


def _host_prep(h, att_feats, internal_att_feats, h2att_w, h2att_b, alpha_w, alpha_b):
    h16 = np.asarray(h, np.float32).astype(np.float16)
    af16 = np.asarray(att_feats, np.float32).astype(np.float16)
    iaf16 = np.asarray(internal_att_feats, np.float32).astype(np.float16)
    h2att_w = np.asarray(h2att_w, np.float32)
    h2att_b = np.asarray(h2att_b, np.float32)
    alpha_w = np.asarray(alpha_w, np.float32)
    alpha_b = np.asarray(alpha_b, np.float32)

    w_t = np.ascontiguousarray(h2att_w.T.astype(np.float16))   # [D, A]
    b_bc = np.tile(h2att_b.reshape(1, A).astype(np.float16), (BPC, 1))
    alpha_bc = np.tile(alpha_w.reshape(1, A).astype(np.float16), (P, 1))
    alphab_bc = np.full((P, 1), float(alpha_b.reshape(-1)[0]), np.float32)

    ind = np.zeros((R, BPC), np.float16)
    rows = np.arange(R)
    ind[rows, rows // L] = 1.0
    ind_t = np.ascontiguousarray(ind.T)                        # [BPC, R]
    ind_tiled = _tile_rows(ind, BPC)                           # [P, NT*BPC]

    in_maps = []
    for i in range(NCORES):
        sl = slice(i * BPC, (i + 1) * BPC)
        h_t = h16[sl].T                                        # [D, BPC]
        in_maps.append(
            {
                "h_t": np.ascontiguousarray(
                    h_t.reshape(KCH, P, BPC).transpose(1, 0, 2).reshape(P, KCH * BPC)
                ),
                "w_t": w_t,
                "b_bc": b_bc,
                "alpha_bc": alpha_bc,
                "alphab_bc": alphab_bc,
                "ind": ind_tiled,
                "ind_t": ind_t,
                "ident": np.eye(P, dtype=np.float16),
                "iaf": _tile_rows(iaf16[sl].reshape(R, A), A),
                "af": _tile_rows(af16[sl].reshape(R, D), D),
            }
        )
    return in_maps


def run(trace=False, **inputs):
    """Run the SPMD kernel; returns (full_output [B, D], BassKernelResults)."""
    nc = _get_program()
    in_maps = _host_prep(**inputs)
    res = run_bass_kernel_spmd(nc, in_maps, list(range(NCORES)), trace=trace)
    out = np.concatenate([res.results[i]["out"] for i in range(NCORES)], axis=0)
    return out, res


def kernel(**inputs):
    out, _ = run(trace=False, **inputs)
    return out


# revision 54
# speedup vs baseline: 1.0980x; 1.0270x over previous
"""Trainium2 Bass kernel for BaseAttention (Bahdanau-style additive attention).

Reference computation (per batch row b):
    att_h  = h @ W.T + b_h                         # [B, A]
    dot    = tanh(iaf + att_h[:, None, :])         # [B, L, A]
    scores = dot @ alpha + alpha_b                 # [B, L]
    w      = softmax(scores, axis=1)               # [B, L]
    out    = sum_l w[b, l] * af[b, l, :]           # [B, D]

Sharding: data-parallel over batch, B=128 -> 16 per core across 8 cores.

The kernel is HBM-bandwidth bound; the big streamed tensors (af, iaf, W) are
downcast to fp16 on the host, halving DMA bytes (rel tolerance is 2e-2; fp16
keeps us ~3e-4).  Per-core layout (rows = (b, l) flattened, R=3136):

  - af and iaf are pre-tiled on the host to [P, NT, *] so every DMA descriptor
    is a large contiguous run (16KB for af groups, 4KB+ for iaf chunks).
  - the tensor engine is power-throttled to ~1.2GHz while DMA streams, so PE
    work during the stream is minimized: per 4-tile af group the 16 N=512
    weighted-sum matmuls are emitted in *reversed* tile order so the first
    matmul's semaphore wait covers the whole group's e-columns (accumulation
    is commutative; start/stop sit on the first/last emitted matmul) and the
    rest run back-to-back with no waits.
  - the softmax denominator needs no per-tile matmul: e-columns are reduced
    over tiles with one DVE op at the end plus a single N=1 matmul.
  - scores via one fused DVE scalar_tensor_tensor: (tanh*1)*alpha with
    accum_out giving the row sums directly; exp batched per group; att_h
    broadcast matmuls staged a group ahead so their PSUM waits resolve off
    the critical path.
"""

from contextlib import ExitStack

import numpy as np

import concourse.bass as bass
import concourse.mybir as mybir
import concourse.tile as tile
from concourse import bacc
from concourse.bass_utils import run_bass_kernel_spmd

F32 = mybir.dt.float32
F16 = mybir.dt.float16
AF_T = mybir.ActivationFunctionType
ALU = mybir.AluOpType

B, L, D, A = 128, 196, 2048, 512
NCORES = 8
BPC = B // NCORES          # 16 batch rows per core
R = BPC * L                # 3136 (b, l) rows per core
P = 128                    # partitions
NT = (R + P - 1) // P      # 25 row tiles (24 full + one 64-row tail)
KCH = D // P               # 16 k-chunks for the h @ W.T matmul
WCH = 4                    # W DMA chunks (earlier att_h start)
DCH = 4                    # d chunks of 512 for the weighted sum
DC = D // DCH              # 512
AFG = 4                    # row tiles per streamed DMA group
TAILR = R - (NT - 1) * P   # 64 rows in the last tile

GROUPS = [(t0, min(AFG, NT - t0)) for t0 in range(0, NT, AFG)]
# iaf chunk issue schedule: {group index -> (tile0, ntiles)}; each chunk must
# be issued before any emit_bc() that reads its tiles (tiles 0-7 go up front)
IAF_CHUNKS = {0: (8, 4), 1: (12, 4), 2: (16, 4), 3: (20, 4), 4: (24, 1)}


def _build_program():
    nc = bacc.Bacc(None, target_bir_lowering=False)

    h_t = nc.declare_dram_parameter("h_t", [P, KCH * BPC], F16, isOutput=False)
    w_t = nc.declare_dram_parameter("w_t", [D, A], F16, isOutput=False)
    b_bc = nc.declare_dram_parameter("b_bc", [BPC, A], F16, isOutput=False)
    alpha_bc = nc.declare_dram_parameter("alpha_bc", [P, A], F16, isOutput=False)
    alphab_bc = nc.declare_dram_parameter("alphab_bc", [P, 1], F32, isOutput=False)
    ind = nc.declare_dram_parameter("ind", [P, NT * BPC], F16, isOutput=False)
    ind_t = nc.declare_dram_parameter("ind_t", [BPC, R], F16, isOutput=False)
    ident = nc.declare_dram_parameter("ident", [P, P], F16, isOutput=False)
    iaf = nc.declare_dram_parameter("iaf", [P, NT * A], F16, isOutput=False)
    af = nc.declare_dram_parameter("af", [P, NT * D], F16, isOutput=False)
    out = nc.declare_dram_parameter("out", [BPC, D], F32, isOutput=True)

    with ExitStack() as ctx:
        tc = ctx.enter_context(tile.TileContext(nc))
        consts = ctx.enter_context(tc.tile_pool(name="consts", bufs=1))
        wpool = ctx.enter_context(tc.tile_pool(name="wpool", bufs=1))
        iafp = ctx.enter_context(tc.tile_pool(name="iafp", bufs=1))
        afp = ctx.enter_context(tc.tile_pool(name="afp", bufs=6))
        scr = ctx.enter_context(tc.tile_pool(name="scr", bufs=4))
        ps_bc = ctx.enter_context(
            tc.tile_pool(name="ps_bc", bufs=3, space=bass.MemorySpace.PSUM)
        )
        ps_hb = ctx.enter_context(
            tc.tile_pool(name="ps_hb", bufs=1, space=bass.MemorySpace.PSUM)
        )
        ps_acc = ctx.enter_context(
            tc.tile_pool(name="ps_acc", bufs=1, space=bass.MemorySpace.PSUM)
        )


        # --- h and W head the DMA queue: att_h gates every scores chain, so
        # its inputs must land first; W in chunks so the accumulation starts
        # while W still streams ---
        ht_sb = consts.tile([P, KCH, BPC], F16)
        nc.sync.dma_start(ht_sb[:], h_t[:, :].rearrange("p (k b) -> p k b", k=KCH))
        w_sb = wpool.tile([P, KCH, A], F16)
        KPW = KCH // WCH
        for wc in range(WCH):
            nc.sync.dma_start(
                w_sb[:, wc * KPW : (wc + 1) * KPW, :],
                w_t[wc * KPW * P : (wc + 1) * KPW * P, :].rearrange(
                    "(k p) a -> p k a", p=P
                ),
            )

        # --- remaining small constants ---
        bbc_sb = consts.tile([BPC, A], F16)
        nc.sync.dma_start(bbc_sb[:], b_bc[:, :])
        indt_sb = consts.tile([BPC, R], F16)
        nc.sync.dma_start(indt_sb[:], ind_t[:, :])
        ident_sb = consts.tile([P, P], F16)
        nc.sync.dma_start(ident_sb[:], ident[:, :])
        abc_sb = consts.tile([P, A], F16)
        nc.sync.dma_start(abc_sb[:], alpha_bc[:, :])
        abb_sb = consts.tile([P, 1], F32)
        nc.sync.dma_start(abb_sb[:], alphab_bc[:, :])
        ind_sb = consts.tile([P, NT, BPC], F16)
        nc.sync.dma_start(ind_sb[:], ind[:, :].rearrange("p (t b) -> p t b", t=NT))

        af_tiles = {}
        iaf_all = iafp.tile([P, NT, A], F16)

        def issue_iaf(t0, n):
            nc.sync.dma_start(
                iaf_all[:, t0 : t0 + n, :],
                iaf[:, t0 * A : (t0 + n) * A].rearrange("p (t a) -> p t a", t=n),
            )

        scores_all = consts.tile([P, NT], F32)
        e_all = consts.tile([P, NT], F32)
        ecols_all = consts.tile([P, NT, BPC], F16)
        # tail-tile rows never written by the scores pipeline must be zero for
        # the end-of-kernel denominator reduce
        nc.vector.memset(ecols_all[TAILR:, NT - 1, :], 0.0)

        # --- att_hb = h @ W.T + b_h, shape [BPC, A] ---
        atthb_ps = ps_hb.tile([BPC, A], F32)
        for k in range(KCH):
            nc.tensor.matmul(
                atthb_ps[:],
                ht_sb[:, k, :],
                w_sb[:, k, :],
                start=(k == 0),
                stop=(k == KCH - 1),
            )
        atthb_sb = consts.tile([BPC, A], F16)
        nc.vector.tensor_add(atthb_sb[:], atthb_ps[:], bbc_sb[:])

        # --- weighted-sum accumulator ---
        acc_ps = ps_acc.tile([BPC, DCH, DC], F32)

        bc_tiles = {}

        def emit_bc(t):
            """x_t = broadcast(att_h) + iaf_t, built fully inside PSUM: a
            row-select matmul then an identity-stationary matmul accumulating
            the iaf tile, so no vector-engine add is needed at all."""
            if t >= NT or t in bc_tiles:
                return
            pt = P if t < NT - 1 else TAILR
            rt = t * P
            bc_ps = ps_bc.tile([P, A], F32, tag="bc")
            nc.tensor.matmul(
                bc_ps[:pt, :],
                indt_sb[:, rt : rt + pt],
                atthb_sb[:],
                start=True,
                stop=False,
            )
            nc.tensor.matmul(
                bc_ps[:pt, :],
                ident_sb[:pt, :pt],
                iaf_all[:pt, t, :],
                start=False,
                stop=True,
            )
            bc_tiles[t] = bc_ps

        issue_iaf(0, 8)
        for t in range(6):
            emit_bc(t)

        # dependency-free filler matmuls into the dead atthb PSUM bank: they
        # keep the tensor engine executing across inter-group idle gaps so its
        # clock stays ramped; the next burst's semaphore wait absorbs them
        fill_ps = atthb_ps

        def emit_fillers(k):
            for _ in range(k):
                nc.tensor.matmul(
                    fill_ps[:], ind_sb[:, 0, :], abc_sb[:, :], start=True, stop=True
                )

        for gi, (t0, n) in enumerate(GROUPS):
            tiles = [(t, P if t < NT - 1 else TAILR) for t in range(t0, t0 + n)]

            # --- stream DMAs ---
            if gi in IAF_CHUNKS:
                issue_iaf(*IAF_CHUNKS[gi])
            af_g = afp.tile([P, AFG, D], F16, tag="af")
            nc.sync.dma_start(
                af_g[:, :n, :],
                af[:, t0 * D : (t0 + n) * D].rearrange("p (t d) -> p t d", t=n),
            )
            for jj in range(n):
                af_tiles[t0 + jj] = (af_g, jj)

            # --- scores chains (scalar + DVE), batched per stage ---
            tanhs = {}
            for t, pt in tiles:
                tanh = scr.tile([P, A], F16, tag="tanh")
                nc.scalar.activation(
                    tanh[:pt, :], bc_tiles.pop(t)[:pt, :], AF_T.Tanh
                )
                tanhs[t] = tanh
            for t, pt in tiles:
                junk = scr.tile([P, A], F16, tag="junk")
                nc.vector.scalar_tensor_tensor(
                    junk[:pt, :],
                    tanhs[t][:pt, :],
                    1.0,
                    abc_sb[:pt, :],
                    op0=ALU.mult,
                    op1=ALU.mult,
                    accum_out=scores_all[:pt, t : t + 1],
                )
            gpt = tiles[-1][1] if t0 + n == NT else P
            nc.scalar.activation(
                e_all[:gpt, t0 : t0 + n],
                scores_all[:gpt, t0 : t0 + n],
                AF_T.Exp,
                bias=abb_sb[:gpt, :],
            )
            for t, pt in tiles:
                nc.vector.tensor_scalar_mul(
                    ecols_all[:pt, t, :], ind_sb[:pt, t, :], e_all[:pt, t : t + 1]
                )

            # the next group's broadcasts all precede the burst (the PSUM
            # rotation frees on this group's tanh reads), so the next scores
            # chain fully overlaps the burst
            for t in range(t0 + 4, t0 + 8):
                emit_bc(t)

            # --- PE: weighted-sum matmuls, reversed so one ecols wait covers
            # the group and the rest run back-to-back ---
            for j, t in enumerate(reversed(range(t0, t0 + n))):
                pt = P if t < NT - 1 else TAILR
                af_g, af_j = af_tiles.pop(t)
                for c in range(DCH):
                    nc.tensor.matmul(
                        acc_ps[:, c, :],
                        ecols_all[:pt, t, :],
                        af_g[:pt, af_j, c * DC : (c + 1) * DC],
                        start=(gi == 0 and j == 0),
                        stop=(t == NT - 1 and c == DCH - 1),
                    )
            # fillers keep the PE clock ramped across early inter-group gaps;
            # late in the stream the chain is the constraint, so skip them
            if gi < 3:
                emit_fillers(3)

        # --- denominator: one reduce over tiles + one N=1 matmul ---
        ered = consts.tile([P, BPC], F32)
        nc.vector.tensor_reduce(
            ered[:, :],
            ecols_all[:, :, :].rearrange("p t b -> p b t"),
            axis=mybir.AxisListType.X,
            op=ALU.add,
        )
        ones1 = consts.tile([P, 1], F32)
        nc.vector.memset(ones1[:], 1.0)
        sums_ps = ps_bc.tile([BPC, 1], F32, tag="bc")
        nc.tensor.matmul(sums_ps[:], ered[:, :], ones1[:], start=True, stop=True)
        recip = consts.tile([BPC, 1], F32)
        nc.vector.reciprocal(recip[:], sums_ps[:])

        # --- normalize (split scalar/vector) and store ---
        out_sb = consts.tile([BPC, D], F32)
        for c in range(DCH):
            dst = out_sb[:, c * DC : (c + 1) * DC]
            if c < 2:
                nc.scalar.mul(dst, acc_ps[:, c, :], recip[:])
            else:
                nc.vector.tensor_scalar_mul(dst, acc_ps[:, c, :], recip[:])
        nc.sync.dma_start(out[:, :], out_sb[:])

    nc.compile()
    return nc


_PROGRAM = None


def _get_program():
    global _PROGRAM
    if _PROGRAM is None:
        _PROGRAM = _build_program()
    return _PROGRAM


def _tile_rows(x, width):
    """[R, width] -> [P, NT*width] with row r at partition r%P, tile r//P."""
    padded = np.zeros((NT * P, width), np.float16)
    padded[:R] = x
    return np.ascontiguousarray(
        padded.reshape(NT, P, width).transpose(1, 0, 2).reshape(P, NT * width)
    )


def _host_prep(h, att_feats, internal_att_feats, h2att_w, h2att_b, alpha_w, alpha_b):
    h16 = np.asarray(h, np.float32).astype(np.float16)
    af16 = np.asarray(att_feats, np.float32).astype(np.float16)
    iaf16 = np.asarray(internal_att_feats, np.float32).astype(np.float16)
    h2att_w = np.asarray(h2att_w, np.float32)
    h2att_b = np.asarray(h2att_b, np.float32)
    alpha_w = np.asarray(alpha_w, np.float32)
    alpha_b = np.asarray(alpha_b, np.float32)

    w_t = np.ascontiguousarray(h2att_w.T.astype(np.float16))   # [D, A]
    b_bc = np.tile(h2att_b.reshape(1, A).astype(np.float16), (BPC, 1))
    alpha_bc = np.tile(alpha_w.reshape(1, A).astype(np.float16), (P, 1))
    alphab_bc = np.full((P, 1), float(alpha_b.reshape(-1)[0]), np.float32)

    ind = np.zeros((R, BPC), np.float16)
    rows = np.arange(R)
    ind[rows, rows // L] = 1.0
    ind_t = np.ascontiguousarray(ind.T)                        # [BPC, R]
    ind_tiled = _tile_rows(ind, BPC)                           # [P, NT*BPC]

    in_maps = []
    for i in range(NCORES):
        sl = slice(i * BPC, (i + 1) * BPC)
        h_t = h16[sl].T                                        # [D, BPC]
        in_maps.append(
            {
                "h_t": np.ascontiguousarray(
                    h_t.reshape(KCH, P, BPC).transpose(1, 0, 2).reshape(P, KCH * BPC)
                ),
                "w_t": w_t,
                "b_bc": b_bc,
                "alpha_bc": alpha_bc,
                "alphab_bc": alphab_bc,
                "ind": ind_tiled,
                "ind_t": ind_t,
                "ident": np.eye(P, dtype=np.float16),
                "iaf": _tile_rows(iaf16[sl].reshape(R, A), A),
                "af": _tile_rows(af16[sl].reshape(R, D), D),
            }
        )
    return in_maps


def run(trace=False, **inputs):
    """Run the SPMD kernel; returns (full_output [B, D], BassKernelResults)."""
    nc = _get_program()
    in_maps = _host_prep(**inputs)
    res = run_bass_kernel_spmd(nc, in_maps, list(range(NCORES)), trace=trace)
    out = np.concatenate([res.results[i]["out"] for i in range(NCORES)], axis=0)
    return out, res


def kernel(**inputs):
    out, _ = run(trace=False, **inputs)
    return out
